# revision 11
# baseline (speedup 1.0000x reference)
"""Trainium2 Bass kernel for nn_BaseDecoder: decode [B, N, 33] f32 tensors of
{sign, 32 base-2 digits} into f32 values via int32 bit packing.

Full inputs [8192, 256, 33] are sharded over 8 NeuronCores along the batch
axis (pure data parallel). Per core: 262144 elements laid out [128, 2048, 33].

Algorithm per core (bit-exact vs the jax reference):
  - A 4-level parallel reduction tree of scalar_tensor_tensor ops
    (out = even*scale + odd with scales 2/4/16/256) packs the 32 digits of
    each element into two exact 16-bit fp32 integers (hi, lo).
  - packed = (int(hi) << 16) | int(lo), then XOR in the sign bit extracted
    from the raw fp32 bits of the sign channel.
"""

from contextlib import ExitStack

import numpy as np

import concourse.bass as bass
import concourse.bacc as bacc
import concourse.tile as tile
from concourse import mybir
from concourse.bass_utils import run_bass_kernel_spmd

P = 128          # SBUF partitions
W = 33           # 1 sign + 32 digits
B, N = 8192, 256
N_CORES = 8
ELEMS = B * N                      # 2097152 total elements
ELEMS_PER_CORE = ELEMS // N_CORES  # 262144
EPP = ELEMS_PER_CORE // P          # 2048 elements per partition
E_TILE = 256                       # elements per partition per tile

F32 = mybir.dt.float32
I32 = mybir.dt.int32
U32 = mybir.dt.uint32
ALU = mybir.AluOpType


def _stt(eng, out, in0, scalar, in1, op0, op1, imm_dtype=F32):
    """scalar_tensor_tensor out = (in0 op0 scalar) op1 in1, with control over
    the immediate dtype (the python wrapper lowers immediates as float32,
    which the walrus verifier rejects for bitvec ops)."""
    return eng.add_instruction(
        mybir.InstTensorScalarPtr(
            name=eng.bass.get_next_instruction_name(),
            is_scalar_tensor_tensor=True,
            op0=op0,
            op1=op1,
            ins=[
                eng.lower_ap(in0),
                mybir.ImmediateValue(dtype=imm_dtype, value=scalar),
                eng.lower_ap(in1),
            ],
            outs=[eng.lower_ap(out)],
        )
    )


def _tile_schedule(epp: int, e_tile: int) -> list[int]:
    """Small tiles at the pipeline fill and drain ends, e_tile in the middle."""
    head = [64, 64, 128]
    tail = [128, 64, 64]
    mid_total = epp - sum(head) - sum(tail)
    assert mid_total >= 0 and mid_total % e_tile == 0
    return head + [e_tile] * (mid_total // e_tile) + tail


def _build_kernel(epp: int, e_tile: int) -> bacc.Bacc:
    assert epp % e_tile == 0
    if epp >= 4 * e_tile and e_tile >= 128:
        schedule = _tile_schedule(epp, e_tile)
    else:
        schedule = [e_tile] * (epp // e_tile)

    nc = bacc.Bacc("TRN2", target_bir_lowering=False, debug=False)

    x = nc.dram_tensor("x", [P, epp, W], F32, kind="ExternalInput")
    y = nc.dram_tensor("y", [P, epp], F32, kind="ExternalOutput")

    x_ap, y_ap = x.ap(), y.ap()

    with tile.TileContext(nc) as tc, ExitStack() as ctx:
        data_pool = ctx.enter_context(tc.tile_pool(name="data", bufs=3))
        mid_pool = ctx.enter_context(tc.tile_pool(name="mid", bufs=2))
        small_pool = ctx.enter_context(tc.tile_pool(name="small", bufs=4))
        out_pool = ctx.enter_context(tc.tile_pool(name="out", bufs=2))
        v = nc.vector

        off = 0
        for E in schedule:
            t0 = off
            off += E
            xt = data_pool.tile([P, E, W], F32, tag="data")
            nc.sync.dma_start(out=xt, in_=x_ap[:, t0 : t0 + E, :])

            # Sign mask from raw fp32 sign-channel bits, emitted first so the
            # data tile's last reader finishes early:
            # (bits << 2) & 0x80000000 maps 1.0 -> 0x80000000, 0.0 -> 0.
            smask = small_pool.tile([P, E], U32, tag="smask")
            v.tensor_scalar(
                out=smask,
                in0=xt[:, :, 0].bitcast(U32),
                scalar1=2,
                scalar2=0x80000000,
                op0=ALU.logical_shift_left,
                op1=ALU.bitwise_and,
            )

            # Reduction tree: digits at positions 1..32 of each 33-group.
            pairs = mid_pool.tile([P, E, 16], F32, tag="pairs")
            _stt(v, pairs, xt[:, :, 1::2], 2.0, xt[:, :, 2::2],
                 op0=ALU.mult, op1=ALU.add)
            quads = mid_pool.tile([P, E, 8], F32, tag="quads")
            _stt(v, quads, pairs[:, :, 0::2], 4.0, pairs[:, :, 1::2],
                 op0=ALU.mult, op1=ALU.add)
            octs = mid_pool.tile([P, E, 4], F32, tag="octs")
            _stt(v, octs, quads[:, :, 0::2], 16.0, quads[:, :, 1::2],
                 op0=ALU.mult, op1=ALU.add)
            # L4 writes int32 directly (fp32 ALU result is an exact integer
            # < 2^16; the output stage converts).
            ihl = mid_pool.tile([P, E, 2], I32, tag="hilo")
            _stt(v, ihl, octs[:, :, 0::2], 256.0, octs[:, :, 1::2],
                 op0=ALU.mult, op1=ALU.add)

            packed = small_pool.tile([P, E], U32, tag="packed")
            _stt(v, packed, ihl[:, :, 0].bitcast(U32), 16,
                 ihl[:, :, 1].bitcast(U32),
                 op0=ALU.logical_shift_left, op1=ALU.bitwise_or,
                 imm_dtype=U32)

            out_t = out_pool.tile([P, E], F32, tag="out")
            v.tensor_tensor(
                out=out_t.bitcast(U32), in0=packed, in1=smask,
                op=ALU.bitwise_xor,
            )

            # Output DMAs go out on the Activation engine's HWDGE queue so
            # their semaphore waits don't head-of-line-block the next input
            # DMA issues on Sync.
            nc.scalar.dma_start(out=y_ap[:, t0 : t0 + E], in_=out_t)

    nc.compile()
    return nc


_NC_CACHE: dict[tuple[int, int], bacc.Bacc] = {}


def _get_nc(epp: int = EPP, e_tile: int = E_TILE) -> bacc.Bacc:
    key = (epp, e_tile)
    if key not in _NC_CACHE:
        _NC_CACHE[key] = _build_kernel(epp, e_tile)
    return _NC_CACHE[key]


def _run(inputs_np: np.ndarray, trace: bool = False):
    """Shard, run on 8 cores, gather. Returns (full_output, BassKernelResults)."""
    nc = _get_nc()
    shards = inputs_np.reshape(N_CORES, P, EPP, W)
    in_maps = [{"x": np.ascontiguousarray(shards[c])} for c in range(N_CORES)]
    res = run_bass_kernel_spmd(
        nc, in_maps, core_ids=list(range(N_CORES)), trace=trace
    )
    out = np.concatenate([r["y"].reshape(-1) for r in res.results])
    return out.reshape(B, N).astype(np.float32, copy=False), res


def kernel(inputs: np.ndarray) -> np.ndarray:
    inputs_np = np.ascontiguousarray(np.asarray(inputs), dtype=np.float32)
    assert inputs_np.shape == (B, N, 1 + 32), inputs_np.shape
    out, _ = _run(inputs_np, trace=False)
    return out


# revision 12
# speedup vs baseline: 1.1428x; 1.1428x over previous
"""Trainium2 Bass kernel for nn_BaseDecoder: decode [B, N, 33] f32 tensors of
{sign, 32 base-2 digits} into f32 values via int32 bit packing.

Full inputs [8192, 256, 33] are sharded over 8 NeuronCores along the batch
axis (pure data parallel). Per core: 262144 elements laid out [128, 2048, 33].

Algorithm per core (bit-exact vs the jax reference):
  - A 4-level parallel reduction tree of scalar_tensor_tensor ops
    (out = even*scale + odd with scales 2/4/16/256) packs the 32 digits of
    each element into two exact 16-bit fp32 integers (hi, lo).
  - packed = (int(hi) << 16) | int(lo), then XOR in the sign bit extracted
    from the raw fp32 bits of the sign channel.
"""

from contextlib import ExitStack

import numpy as np

import concourse.bass as bass
import concourse.bacc as bacc
import concourse.tile as tile
from concourse import mybir
from concourse.bass_utils import run_bass_kernel_spmd

P = 128          # SBUF partitions
W = 33           # 1 sign + 32 digits
B, N = 8192, 256
N_CORES = 8
ELEMS = B * N                      # 2097152 total elements
ELEMS_PER_CORE = ELEMS // N_CORES  # 262144
EPP = ELEMS_PER_CORE // P          # 2048 elements per partition
E_TILE = 256                       # elements per partition per tile

F32 = mybir.dt.float32
I32 = mybir.dt.int32
U32 = mybir.dt.uint32
ALU = mybir.AluOpType


def _stt(eng, out, in0, scalar, in1, op0, op1, imm_dtype=F32):
    """scalar_tensor_tensor out = (in0 op0 scalar) op1 in1, with control over
    the immediate dtype (the python wrapper lowers immediates as float32,
    which the walrus verifier rejects for bitvec ops)."""
    return eng.add_instruction(
        mybir.InstTensorScalarPtr(
            name=eng.bass.get_next_instruction_name(),
            is_scalar_tensor_tensor=True,
            op0=op0,
            op1=op1,
            ins=[
                eng.lower_ap(in0),
                mybir.ImmediateValue(dtype=imm_dtype, value=scalar),
                eng.lower_ap(in1),
            ],
            outs=[eng.lower_ap(out)],
        )
    )


def _tile_schedule(epp: int, e_tile: int) -> list[int]:
    """Small tiles at the pipeline fill and drain ends, e_tile in the middle."""
    head = [64, 64, 128]
    tail = [128, 64, 64]
    mid_total = epp - sum(head) - sum(tail)
    assert mid_total >= 0 and mid_total % e_tile == 0
    return head + [e_tile] * (mid_total // e_tile) + tail


def _build_kernel(epp: int, e_tile: int) -> bacc.Bacc:
    assert epp % e_tile == 0
    schedule = [e_tile] * (epp // e_tile)

    nc = bacc.Bacc("TRN2", target_bir_lowering=False, debug=False)

    x = nc.dram_tensor("x", [P, epp, W], F32, kind="ExternalInput")
    y = nc.dram_tensor("y", [P, epp], F32, kind="ExternalOutput")

    x_ap, y_ap = x.ap(), y.ap()

    with tile.TileContext(nc) as tc, ExitStack() as ctx:
        data_pool = ctx.enter_context(tc.tile_pool(name="data", bufs=3))
        mid_pool = ctx.enter_context(tc.tile_pool(name="mid", bufs=2))
        small_pool = ctx.enter_context(tc.tile_pool(name="small", bufs=4))
        out_pool = ctx.enter_context(tc.tile_pool(name="out", bufs=2))
        v = nc.vector

        off = 0
        for E in schedule:
            t0 = off
            off += E
            xt = data_pool.tile([P, E, W], F32, tag="data")
            nc.sync.dma_start(out=xt, in_=x_ap[:, t0 : t0 + E, :])

            # Sign mask from raw fp32 sign-channel bits, emitted first so the
            # data tile's last reader finishes early:
            # (bits << 2) & 0x80000000 maps 1.0 -> 0x80000000, 0.0 -> 0.
            smask = small_pool.tile([P, E], U32, tag="smask")
            v.tensor_scalar(
                out=smask,
                in0=xt[:, :, 0].bitcast(U32),
                scalar1=2,
                scalar2=0x80000000,
                op0=ALU.logical_shift_left,
                op1=ALU.bitwise_and,
            )

            # Reduction tree: digits at positions 1..32 of each 33-group.
            pairs = mid_pool.tile([P, E, 16], F32, tag="pairs")
            _stt(v, pairs, xt[:, :, 1::2], 2.0, xt[:, :, 2::2],
                 op0=ALU.mult, op1=ALU.add)
            quads = mid_pool.tile([P, E, 8], F32, tag="quads")
            _stt(v, quads, pairs[:, :, 0::2], 4.0, pairs[:, :, 1::2],
                 op0=ALU.mult, op1=ALU.add)
            octs = mid_pool.tile([P, E, 4], F32, tag="octs")
            _stt(v, octs, quads[:, :, 0::2], 16.0, quads[:, :, 1::2],
                 op0=ALU.mult, op1=ALU.add)
            # L4 writes int32 directly (fp32 ALU result is an exact integer
            # < 2^16; the output stage converts).
            ihl = mid_pool.tile([P, E, 2], I32, tag="hilo")
            _stt(v, ihl, octs[:, :, 0::2], 256.0, octs[:, :, 1::2],
                 op0=ALU.mult, op1=ALU.add)

            packed = small_pool.tile([P, E], U32, tag="packed")
            _stt(v, packed, ihl[:, :, 0].bitcast(U32), 16,
                 ihl[:, :, 1].bitcast(U32),
                 op0=ALU.logical_shift_left, op1=ALU.bitwise_or,
                 imm_dtype=U32)

            out_t = out_pool.tile([P, E], F32, tag="out")
            v.tensor_tensor(
                out=out_t.bitcast(U32), in0=packed, in1=smask,
                op=ALU.bitwise_xor,
            )

            # Output DMAs go out on the Activation engine's HWDGE queue so
            # their semaphore waits don't head-of-line-block the next input
            # DMA issues on Sync.
            nc.scalar.dma_start(out=y_ap[:, t0 : t0 + E], in_=out_t)

    nc.compile()
    return nc


_NC_CACHE: dict[tuple[int, int], bacc.Bacc] = {}


def _get_nc(epp: int = EPP, e_tile: int = E_TILE) -> bacc.Bacc:
    key = (epp, e_tile)
    if key not in _NC_CACHE:
        _NC_CACHE[key] = _build_kernel(epp, e_tile)
    return _NC_CACHE[key]


def _run(inputs_np: np.ndarray, trace: bool = False):
    """Shard, run on 8 cores, gather. Returns (full_output, BassKernelResults)."""
    nc = _get_nc()
    shards = inputs_np.reshape(N_CORES, P, EPP, W)
    in_maps = [{"x": np.ascontiguousarray(shards[c])} for c in range(N_CORES)]
    res = run_bass_kernel_spmd(
        nc, in_maps, core_ids=list(range(N_CORES)), trace=trace
    )
    out = np.concatenate([r["y"].reshape(-1) for r in res.results])
    return out.reshape(B, N).astype(np.float32, copy=False), res


def kernel(inputs: np.ndarray) -> np.ndarray:
    inputs_np = np.ascontiguousarray(np.asarray(inputs), dtype=np.float32)
    assert inputs_np.shape == (B, N, 1 + 32), inputs_np.shape
    out, _ = _run(inputs_np, trace=False)
    return out


# revision 13
# speedup vs baseline: 1.1470x; 1.0036x over previous
"""Trainium2 Bass kernel for nn_BaseDecoder: decode [B, N, 33] f32 tensors of
{sign, 32 base-2 digits} into f32 values via int32 bit packing.

Full inputs [8192, 256, 33] are sharded over 8 NeuronCores along the batch
axis (pure data parallel). Per core: 262144 elements laid out [128, 2048, 33].

Algorithm per core (bit-exact vs the jax reference):
  - A 4-level parallel reduction tree of scalar_tensor_tensor ops
    (out = even*scale + odd with scales 2/4/16/256) packs the 32 digits of
    each element into two exact 16-bit fp32 integers (hi, lo).
  - packed = (int(hi) << 16) | int(lo), then XOR in the sign bit extracted
    from the raw fp32 bits of the sign channel.
"""

from contextlib import ExitStack

import numpy as np

import concourse.bass as bass
import concourse.bacc as bacc
import concourse.tile as tile
from concourse import mybir
from concourse.bass_utils import run_bass_kernel_spmd

P = 128          # SBUF partitions
W = 33           # 1 sign + 32 digits
B, N = 8192, 256
N_CORES = 8
ELEMS = B * N                      # 2097152 total elements
ELEMS_PER_CORE = ELEMS // N_CORES  # 262144
EPP = ELEMS_PER_CORE // P          # 2048 elements per partition
E_TILE = 256                       # elements per partition per tile

F32 = mybir.dt.float32
I32 = mybir.dt.int32
U32 = mybir.dt.uint32
ALU = mybir.AluOpType


def _stt(eng, out, in0, scalar, in1, op0, op1, imm_dtype=F32):
    """scalar_tensor_tensor out = (in0 op0 scalar) op1 in1, with control over
    the immediate dtype (the python wrapper lowers immediates as float32,
    which the walrus verifier rejects for bitvec ops)."""
    return eng.add_instruction(
        mybir.InstTensorScalarPtr(
            name=eng.bass.get_next_instruction_name(),
            is_scalar_tensor_tensor=True,
            op0=op0,
            op1=op1,
            ins=[
                eng.lower_ap(in0),
                mybir.ImmediateValue(dtype=imm_dtype, value=scalar),
                eng.lower_ap(in1),
            ],
            outs=[eng.lower_ap(out)],
        )
    )


def _tile_schedule(epp: int, e_tile: int) -> list[int]:
    """Uniform tiles with a tapered tail: the last tile's compute chain sits
    on the critical path after the final DMA bytes land, so shrink it."""
    tail = [e_tile // 2, 3 * e_tile // 8, e_tile // 8]
    mid_total = epp - sum(tail)
    if mid_total <= 0 or mid_total % e_tile or e_tile < 128:
        return [e_tile] * (epp // e_tile)
    return [e_tile] * (mid_total // e_tile) + tail


def _build_kernel(epp: int, e_tile: int) -> bacc.Bacc:
    assert epp % e_tile == 0
    schedule = _tile_schedule(epp, e_tile)

    nc = bacc.Bacc("TRN2", target_bir_lowering=False, debug=False)

    x = nc.dram_tensor("x", [P, epp, W], F32, kind="ExternalInput")
    y = nc.dram_tensor("y", [P, epp], F32, kind="ExternalOutput")

    x_ap, y_ap = x.ap(), y.ap()

    with tile.TileContext(nc) as tc, ExitStack() as ctx:
        data_pool = ctx.enter_context(tc.tile_pool(name="data", bufs=4))
        mid_pool = ctx.enter_context(tc.tile_pool(name="mid", bufs=1))
        small_pool = ctx.enter_context(tc.tile_pool(name="small", bufs=4))
        out_pool = ctx.enter_context(tc.tile_pool(name="out", bufs=2))
        v = nc.vector

        off = 0
        for E in schedule:
            t0 = off
            off += E
            xt = data_pool.tile([P, E, W], F32, tag="data")
            nc.sync.dma_start(out=xt, in_=x_ap[:, t0 : t0 + E, :])

            # Sign mask from raw fp32 sign-channel bits, emitted first so the
            # data tile's last reader finishes early:
            # (bits << 2) & 0x80000000 maps 1.0 -> 0x80000000, 0.0 -> 0.
            smask = small_pool.tile([P, E], U32, tag="smask")
            v.tensor_scalar(
                out=smask,
                in0=xt[:, :, 0].bitcast(U32),
                scalar1=2,
                scalar2=0x80000000,
                op0=ALU.logical_shift_left,
                op1=ALU.bitwise_and,
            )

            # Reduction tree: digits at positions 1..32 of each 33-group.
            pairs = mid_pool.tile([P, E, 16], F32, tag="pairs")
            _stt(v, pairs, xt[:, :, 1::2], 2.0, xt[:, :, 2::2],
                 op0=ALU.mult, op1=ALU.add)
            quads = mid_pool.tile([P, E, 8], F32, tag="quads")
            _stt(v, quads, pairs[:, :, 0::2], 4.0, pairs[:, :, 1::2],
                 op0=ALU.mult, op1=ALU.add)
            octs = mid_pool.tile([P, E, 4], F32, tag="octs")
            _stt(v, octs, quads[:, :, 0::2], 16.0, quads[:, :, 1::2],
                 op0=ALU.mult, op1=ALU.add)
            # L4 writes int32 directly (fp32 ALU result is an exact integer
            # < 2^16; the output stage converts).
            ihl = mid_pool.tile([P, E, 2], I32, tag="hilo")
            _stt(v, ihl, octs[:, :, 0::2], 256.0, octs[:, :, 1::2],
                 op0=ALU.mult, op1=ALU.add)

            packed = small_pool.tile([P, E], U32, tag="packed")
            _stt(v, packed, ihl[:, :, 0].bitcast(U32), 16,
                 ihl[:, :, 1].bitcast(U32),
                 op0=ALU.logical_shift_left, op1=ALU.bitwise_or,
                 imm_dtype=U32)

            out_t = out_pool.tile([P, E], F32, tag="out")
            v.tensor_tensor(
                out=out_t.bitcast(U32), in0=packed, in1=smask,
                op=ALU.bitwise_xor,
            )

            # Output DMAs go out on the Activation engine's HWDGE queue so
            # their semaphore waits don't head-of-line-block the next input
            # DMA issues on Sync.
            nc.scalar.dma_start(out=y_ap[:, t0 : t0 + E], in_=out_t)

    nc.compile()
    return nc


_NC_CACHE: dict[tuple[int, int], bacc.Bacc] = {}


def _get_nc(epp: int = EPP, e_tile: int = E_TILE) -> bacc.Bacc:
    key = (epp, e_tile)
    if key not in _NC_CACHE:
        _NC_CACHE[key] = _build_kernel(epp, e_tile)
    return _NC_CACHE[key]


def _run(inputs_np: np.ndarray, trace: bool = False):
    """Shard, run on 8 cores, gather. Returns (full_output, BassKernelResults)."""
    nc = _get_nc()
    shards = inputs_np.reshape(N_CORES, P, EPP, W)
    in_maps = [{"x": np.ascontiguousarray(shards[c])} for c in range(N_CORES)]
    res = run_bass_kernel_spmd(
        nc, in_maps, core_ids=list(range(N_CORES)), trace=trace
    )
    out = np.concatenate([r["y"].reshape(-1) for r in res.results])
    return out.reshape(B, N).astype(np.float32, copy=False), res


def kernel(inputs: np.ndarray) -> np.ndarray:
    inputs_np = np.ascontiguousarray(np.asarray(inputs), dtype=np.float32)
    assert inputs_np.shape == (B, N, 1 + 32), inputs_np.shape
    out, _ = _run(inputs_np, trace=False)
    return out


# revision 14
# speedup vs baseline: 1.1511x; 1.0036x over previous
"""Trainium2 Bass kernel for nn_BaseDecoder: decode [B, N, 33] f32 tensors of
{sign, 32 base-2 digits} into f32 values via int32 bit packing.

Full inputs [8192, 256, 33] are sharded over 8 NeuronCores along the batch
axis (pure data parallel). Per core: 262144 elements laid out [128, 2048, 33].

Algorithm per core (bit-exact vs the jax reference):
  - A 4-level parallel reduction tree of scalar_tensor_tensor ops
    (out = even*scale + odd with scales 2/4/16/256) packs the 32 digits of
    each element into two exact 16-bit fp32 integers (hi, lo).
  - packed = (int(hi) << 16) | int(lo), then XOR in the sign bit extracted
    from the raw fp32 bits of the sign channel.
"""

from contextlib import ExitStack

import numpy as np

import concourse.bass as bass
import concourse.bacc as bacc
import concourse.tile as tile
from concourse import mybir
from concourse.bass_utils import run_bass_kernel_spmd

P = 128          # SBUF partitions
W = 33           # 1 sign + 32 digits
B, N = 8192, 256
N_CORES = 8
ELEMS = B * N                      # 2097152 total elements
ELEMS_PER_CORE = ELEMS // N_CORES  # 262144
EPP = ELEMS_PER_CORE // P          # 2048 elements per partition
E_TILE = 256                       # elements per partition per tile

F32 = mybir.dt.float32
I32 = mybir.dt.int32
U32 = mybir.dt.uint32
ALU = mybir.AluOpType


def _stt(eng, out, in0, scalar, in1, op0, op1, imm_dtype=F32):
    """scalar_tensor_tensor out = (in0 op0 scalar) op1 in1, with control over
    the immediate dtype (the python wrapper lowers immediates as float32,
    which the walrus verifier rejects for bitvec ops)."""
    return eng.add_instruction(
        mybir.InstTensorScalarPtr(
            name=eng.bass.get_next_instruction_name(),
            is_scalar_tensor_tensor=True,
            op0=op0,
            op1=op1,
            ins=[
                eng.lower_ap(in0),
                mybir.ImmediateValue(dtype=imm_dtype, value=scalar),
                eng.lower_ap(in1),
            ],
            outs=[eng.lower_ap(out)],
        )
    )


def _tile_schedule(epp: int, e_tile: int) -> list[int]:
    """Tapered head and tail around uniform middle tiles. Head: small first
    tiles land sooner (the 4-deep prefetch divides bandwidth, so a full-size
    first tile starts the vector engine ~10us late, and fast-mode runs are
    vector-bound). Tail: the last tile's compute chain sits after the final
    DMA bytes, so shrink it."""
    head = [e_tile // 8, e_tile // 4, 5 * e_tile // 8]
    tail = [e_tile // 2, 3 * e_tile // 8, e_tile // 8]
    mid_total = epp - sum(head) - sum(tail)
    if mid_total <= 0 or mid_total % e_tile or e_tile < 128:
        return [e_tile] * (epp // e_tile)
    return head + [e_tile] * (mid_total // e_tile) + tail


def _build_kernel(epp: int, e_tile: int) -> bacc.Bacc:
    assert epp % e_tile == 0
    schedule = _tile_schedule(epp, e_tile)

    nc = bacc.Bacc("TRN2", target_bir_lowering=False, debug=False)

    x = nc.dram_tensor("x", [P, epp, W], F32, kind="ExternalInput")
    y = nc.dram_tensor("y", [P, epp], F32, kind="ExternalOutput")

    x_ap, y_ap = x.ap(), y.ap()

    with tile.TileContext(nc) as tc, ExitStack() as ctx:
        data_pool = ctx.enter_context(tc.tile_pool(name="data", bufs=4))
        mid_pool = ctx.enter_context(tc.tile_pool(name="mid", bufs=1))
        small_pool = ctx.enter_context(tc.tile_pool(name="small", bufs=4))
        out_pool = ctx.enter_context(tc.tile_pool(name="out", bufs=2))
        v = nc.vector

        off = 0
        for E in schedule:
            t0 = off
            off += E
            xt = data_pool.tile([P, E, W], F32, tag="data")
            nc.sync.dma_start(out=xt, in_=x_ap[:, t0 : t0 + E, :])

            # Sign mask from raw fp32 sign-channel bits, emitted first so the
            # data tile's last reader finishes early:
            # (bits << 2) & 0x80000000 maps 1.0 -> 0x80000000, 0.0 -> 0.
            smask = small_pool.tile([P, E], U32, tag="smask")
            v.tensor_scalar(
                out=smask,
                in0=xt[:, :, 0].bitcast(U32),
                scalar1=2,
                scalar2=0x80000000,
                op0=ALU.logical_shift_left,
                op1=ALU.bitwise_and,
            )

            # Reduction tree: digits at positions 1..32 of each 33-group.
            pairs = mid_pool.tile([P, E, 16], F32, tag="pairs")
            _stt(v, pairs, xt[:, :, 1::2], 2.0, xt[:, :, 2::2],
                 op0=ALU.mult, op1=ALU.add)
            quads = mid_pool.tile([P, E, 8], F32, tag="quads")
            _stt(v, quads, pairs[:, :, 0::2], 4.0, pairs[:, :, 1::2],
                 op0=ALU.mult, op1=ALU.add)
            octs = mid_pool.tile([P, E, 4], F32, tag="octs")
            _stt(v, octs, quads[:, :, 0::2], 16.0, quads[:, :, 1::2],
                 op0=ALU.mult, op1=ALU.add)
            # L4 writes int32 directly (fp32 ALU result is an exact integer
            # < 2^16; the output stage converts).
            ihl = mid_pool.tile([P, E, 2], I32, tag="hilo")
            _stt(v, ihl, octs[:, :, 0::2], 256.0, octs[:, :, 1::2],
                 op0=ALU.mult, op1=ALU.add)

            packed = small_pool.tile([P, E], U32, tag="packed")
            _stt(v, packed, ihl[:, :, 0].bitcast(U32), 16,
                 ihl[:, :, 1].bitcast(U32),
                 op0=ALU.logical_shift_left, op1=ALU.bitwise_or,
                 imm_dtype=U32)

            out_t = out_pool.tile([P, E], F32, tag="out")
            v.tensor_tensor(
                out=out_t.bitcast(U32), in0=packed, in1=smask,
                op=ALU.bitwise_xor,
            )

            # Output DMAs go out on the Activation engine's HWDGE queue so
            # their semaphore waits don't head-of-line-block the next input
            # DMA issues on Sync.
            nc.scalar.dma_start(out=y_ap[:, t0 : t0 + E], in_=out_t)

    nc.compile()
    return nc


_NC_CACHE: dict[tuple[int, int], bacc.Bacc] = {}


def _get_nc(epp: int = EPP, e_tile: int = E_TILE) -> bacc.Bacc:
    key = (epp, e_tile)
    if key not in _NC_CACHE:
        _NC_CACHE[key] = _build_kernel(epp, e_tile)
    return _NC_CACHE[key]


def _run(inputs_np: np.ndarray, trace: bool = False):
    """Shard, run on 8 cores, gather. Returns (full_output, BassKernelResults)."""
    nc = _get_nc()
    shards = inputs_np.reshape(N_CORES, P, EPP, W)
    in_maps = [{"x": np.ascontiguousarray(shards[c])} for c in range(N_CORES)]
    res = run_bass_kernel_spmd(
        nc, in_maps, core_ids=list(range(N_CORES)), trace=trace
    )
    out = np.concatenate([r["y"].reshape(-1) for r in res.results])
    return out.reshape(B, N).astype(np.float32, copy=False), res


def kernel(inputs: np.ndarray) -> np.ndarray:
    inputs_np = np.ascontiguousarray(np.asarray(inputs), dtype=np.float32)
    assert inputs_np.shape == (B, N, 1 + 32), inputs_np.shape
    out, _ = _run(inputs_np, trace=False)
    return out
